# revision 4
# baseline (speedup 1.0000x reference)
"""Trainium2 Bass kernel for single-head self-attention over image tokens.

Reference computation (per batch element b of 4):
    xf   = x[b] viewed as [N=4096 tokens, C=256]          (x stored [C, H*W] = xf.T)
    qkv  = xf @ w_qkv.T                                   -> q, k, v each [N, 512]
    sim  = (q * 64**-0.5) @ k.T                           [N, N]
    attn = softmax(sim, axis=-1)
    out  = (attn @ v) @ w_out.T + b_out + xf              [N, C]

Sharding: 8 cores = 4 batches x 2 query-row halves (2048 rows each). Each core
computes k/v for its full batch but q/out only for its half. No collectives.
Each core's x is host-rotated so its query half is always columns 0:2048
(softmax over keys is permutation invariant, so key order doesn't matter).

On-chip layout keeps everything in the "transposed activation" orientation so
no PE transposes are needed:
    qT [512, 2048] and kT [512, N] come straight out of the QKV projection
    (x's HBM layout [C, N] is already the rhs/lhsT the PE wants);
    v [N, 512] comes from the same projection with x slices as the stationary
    operand. simT [j, i] = kT.T @ qT, pT = exp(0.125*simT), then
    outT [d, i] += v_j.T @ pT and l[1, i] += ones.T @ pT accumulate in PSUM
    per 1024-column j-superblock. Softmax normalization is folded in at the
    end: recip(l) via exp(-ln(l)), broadcast across partitions with a K=1
    rank-1 matmul, multiplied into the final projection output.

Matmul operands use float32r (fp32 data rounded/packed by a DVE/ACT copy or
produced directly by activations): 1 PE column/cycle vs 4 for plain fp32.
"""

import numpy as np

import concourse.bacc as bacc
import concourse.tile as tile
import concourse.mybir as mybir
from concourse.bass_utils import run_bass_kernel_spmd

F32 = mybir.dt.float32
F32R = mybir.dt.float32r
Exp = mybir.ActivationFunctionType.Exp
Ln = mybir.ActivationFunctionType.Ln

B = 4
C = 256          # model dim (2 chunks of 128)
N = 4096         # tokens per batch (64*64)
HALF = N // 2    # query rows per core
INNER = 512      # qkv inner dim (4 chunks of 128)
SCALE = 0.125    # 64 ** -0.5

NCORES = 8
NJB = 4          # j superblocks per batch
JBW = N // NJB   # 1024 key columns per superblock
NSL = 4          # i slices per core
SW = HALF // NSL # 512 query columns per slice


def build_nc(n=N, njb=NJB, nsl=NSL):
    half = n // 2
    jbw = n // njb
    assert half % SW == 0 and jbw % SW == 0 and jbw % 128 == 0
    nc = bacc.Bacc(None)
    x_full = nc.declare_dram_parameter("x_full", [C, n], F32, isOutput=False)
    wqkvT = nc.declare_dram_parameter("wqkvT", [C, 3 * INNER], F32, isOutput=False)
    woutT = nc.declare_dram_parameter("woutT", [INNER, C], F32, isOutput=False)
    bout = nc.declare_dram_parameter("bout", [2, 128, 1], F32, isOutput=False)
    out = nc.declare_dram_parameter("out", [C, half], F32, isOutput=True)

    mm = nc.tensor.matmul

    with tile.TileContext(nc) as tc:
        with tc.tile_pool(name="const", bufs=1) as const, \
             tc.tile_pool(name="stream", bufs=1) as stream, \
             tc.tile_pool(name="work", bufs=2) as work, \
             tc.tile_pool(name="pp", bufs=1, space="PSUM") as pp:

            def stage_cast(dst_r, src_ap, cols, chunk=1024):
                """DMA f32 HBM -> staging, DVE-convert into f32r tile dst_r."""
                o = 0
                while o < cols:
                    w = min(chunk, cols - o)
                    stg = stream.tile([128, w], F32, tag="stg", bufs=3,
                                      name="stg", padded_shape=[128, 1024])
                    nc.sync.dma_start(stg, src_ap[:, o:o + w])
                    nc.vector.tensor_copy(dst_r[:, o:o + w], stg)
                    o += w

            # ---- resident weights (f32r) ----
            wq = []
            for cc in range(2):
                t = const.tile([128, 3 * INNER], F32R, tag=f"wq{cc}", name=f"wq{cc}")
                stage_cast(t, wqkvT[cc * 128:(cc + 1) * 128, :], 3 * INNER, chunk=768)
                wq.append(t)
            wo = []
            for d in range(4):
                t = const.tile([128, C], F32R, tag=f"wo{d}", name=f"wo{d}")
                stage_cast(t, woutT[d * 128:(d + 1) * 128, :], C)
                wo.append(t)
            # residual x (+bias) in plain f32, query half = x columns 0:HALF
            xqt = []
            for cc in range(2):
                t = const.tile([128, half], F32, tag=f"xq{cc}", name=f"xq{cc}")
                nc.sync.dma_start(t, x_full[cc * 128:(cc + 1) * 128, 0:half])
                xqt.append(t)
            bt = []
            for cc in range(2):
                t = const.tile([128, 1], F32, tag=f"b{cc}", name=f"b{cc}")
                nc.sync.dma_start(t, bout[cc])
                bt.append(t)
            ones_col_f = const.tile([128, 1], F32, tag="ones_col_f", name="ones_col_f")
            nc.vector.memset(ones_col_f, 1.0)
            ones_col = const.tile([128, 1], F32R, tag="ones_col", name="ones_col")
            nc.vector.tensor_copy(ones_col, ones_col_f)
            ones_row_f = const.tile([1, 128], F32, tag="ones_row_f", name="ones_row_f")
            nc.vector.memset(ones_row_f, 1.0)
            ones_row = const.tile([1, 128], F32R, tag="ones_row", name="ones_row")
            nc.vector.tensor_copy(ones_row, ones_row_f)

            qT = [const.tile([128, half], F32R, tag=f"qt{d}", name=f"qt{d}")
                  for d in range(4)]
            ot = [const.tile([128, half], F32, tag=f"ot{d}", name=f"ot{d}")
                  for d in range(4)]
            l_sb = const.tile([1, half], F32, tag="l_sb", name="l_sb")

            # ---- qT production from x columns 0:HALF ----
            qcw = min(1024, half)
            for qch in range(half // qcw):  # 1024-col chunks of the query half
                xch = []
                for cc in range(2):
                    t = stream.tile([128, qcw], F32R, tag="stg_r", bufs=2,
                                    name="xch", padded_shape=[128, 1024])
                    stage_cast(t, x_full[cc * 128:(cc + 1) * 128,
                                         qch * qcw:(qch + 1) * qcw], qcw)
                    xch.append(t)
                for d in range(4):
                    for nb in range(qcw // SW):
                        ns = qch * (qcw // SW) + nb
                        ps = pp.tile([128, SW], F32, tag="sim", bufs=2, name="ps_q")
                        for cc in range(2):
                            mm(ps, wq[cc][:, d * 128:(d + 1) * 128],
                               xch[cc][:, nb * SW:(nb + 1) * SW],
                               start=(cc == 0), stop=(cc == 1))
                        nc.scalar.copy(qT[d][:, ns * SW:(ns + 1) * SW], ps)

            # residual-with-bias: xqt <- xqt + b
            for cc in range(2):
                nc.vector.tensor_scalar_add(xqt[cc], xqt[cc], bt[cc])

            # ---- attention over j superblocks ----
            for jb in range(njb):
                xjb = []
                for cc in range(2):
                    t = stream.tile([128, jbw], F32R, tag=f"xjb{cc}", bufs=1,
                                    name=f"xjb{cc}")
                    stage_cast(t, x_full[cc * 128:(cc + 1) * 128,
                                         jb * jbw:(jb + 1) * jbw], jbw)
                    xjb.append(t)
                # kT for this superblock: [512, JBW]
                kt = [stream.tile([128, jbw], F32R, tag=f"kt{d}", bufs=1,
                                  name=f"kt{d}") for d in range(4)]
                for d in range(4):
                    for nb in range(jbw // SW):
                        ps = pp.tile([128, SW], F32, tag="sim", bufs=2, name="ps_k")
                        for cc in range(2):
                            mm(ps, wq[cc][:, INNER + d * 128:INNER + (d + 1) * 128],
                               xjb[cc][:, nb * SW:(nb + 1) * SW],
                               start=(cc == 0), stop=(cc == 1))
                        nc.scalar.copy(kt[d][:, nb * SW:(nb + 1) * SW], ps)
                # v for this superblock: [JBW, 512] (token rows on partitions)
                vt = []
                for nj in range(jbw // 128):
                    t = stream.tile([128, INNER], F32R, tag=f"vt{nj}", bufs=1,
                                    name=f"vt{nj}")
                    ps = pp.tile([128, INNER], F32, tag="sim", bufs=2, name="ps_v")
                    for cc in range(2):
                        mm(ps, xjb[cc][:, nj * 128:(nj + 1) * 128],
                           wq[cc][:, 2 * INNER:3 * INNER],
                           start=(cc == 0), stop=(cc == 1))
                    nc.scalar.copy(t, ps)
                    vt.append(t)

                for s in range(nsl):
                    sl = slice(s * SW, (s + 1) * SW)
                    po = [pp.tile([128, SW], F32, tag=f"po{d}", bufs=1,
                                  name=f"po{d}") for d in range(4)]
                    pl = pp.tile([1, SW], F32, tag="aux", bufs=2, name="pl")
                    for j8 in range(jbw // 128):
                        ps = pp.tile([128, SW], F32, tag="sim", bufs=2, name="ps_s")
                        for d in range(4):
                            mm(ps, kt[d][:, j8 * 128:(j8 + 1) * 128], qT[d][:, sl],
                               start=(d == 0), stop=(d == 3))
                        pt = work.tile([128, SW], F32R, tag="pt", bufs=3, name="pt")
                        nc.scalar.activation(pt, ps, Exp, scale=SCALE)
                        for d in range(4):
                            mm(po[d], vt[j8][:, d * 128:(d + 1) * 128], pt,
                               start=(j8 == 0), stop=(j8 == jbw // 128 - 1))
                        mm(pl, ones_col, pt,
                           start=(j8 == 0), stop=(j8 == jbw // 128 - 1))
                    for d in range(4):
                        if jb == 0:
                            nc.vector.tensor_copy(ot[d][:, sl], po[d])
                        else:
                            nc.vector.tensor_add(ot[d][:, sl], ot[d][:, sl], po[d])
                    if jb == 0:
                        nc.vector.tensor_copy(l_sb[:, sl], pl)
                    else:
                        nc.vector.tensor_add(l_sb[:, sl], l_sb[:, sl], pl)

            # ---- softmax normalizer: recip(l) = exp(-ln(l)), f32r for matmul ----
            nc.scalar.activation(l_sb, l_sb, Ln)
            l_r = const.tile([1, half], F32R, tag="l_r", name="l_r")
            nc.scalar.activation(l_r, l_sb, Exp, scale=-1.0)

            # ---- output projection + normalize + bias/residual ----
            for s in range(nsl):
                sl = slice(s * SW, (s + 1) * SW)
                pb = pp.tile([128, SW], F32, tag="aux", bufs=2, name="pb")
                mm(pb, ones_row, l_r[:, sl], start=True, stop=True)
                bc = work.tile([128, SW], F32, tag="bc", bufs=2, name="bc")
                nc.scalar.copy(bc, pb)
                otr = [work.tile([128, SW], F32R, tag=f"otr{d}", bufs=1,
                                 name=f"otr{d}") for d in range(4)]
                for d in range(4):
                    nc.vector.tensor_copy(otr[d], ot[d][:, sl])
                for cc in range(2):
                    pf = pp.tile([128, SW], F32, tag="aux", bufs=2, name="pf")
                    for d in range(4):
                        mm(pf, wo[d][:, cc * 128:(cc + 1) * 128], otr[d],
                           start=(d == 0), stop=(d == 3))
                    fo = work.tile([128, SW], F32, tag="fo", bufs=3, name="fo")
                    nc.vector.tensor_mul(fo, pf, bc)
                    nc.vector.tensor_add(fo, fo, xqt[cc][:, sl])
                    nc.sync.dma_start(out[cc * 128:(cc + 1) * 128, sl], fo)

    nc.finalize()
    return nc


_NC_CACHE = None


def _get_nc():
    global _NC_CACHE
    if _NC_CACHE is None:
        _NC_CACHE = build_nc()
    return _NC_CACHE


def prepare_in_maps(x, w_qkv, w_out, b_out):
    x = np.asarray(x, dtype=np.float32)
    w_qkv = np.asarray(w_qkv, dtype=np.float32)
    w_out = np.asarray(w_out, dtype=np.float32)
    b_out = np.asarray(b_out, dtype=np.float32)

    xr = x.reshape(B, C, N)
    wqkvT = np.ascontiguousarray(w_qkv.T)          # [C, 1536]
    woutT = np.ascontiguousarray(w_out.T)          # [512, C]
    bout = np.ascontiguousarray(b_out.reshape(2, 128, 1))

    in_maps = []
    for c in range(NCORES):
        b, h = divmod(c, 2)
        if h == 0:
            x_rot = xr[b]
        else:  # rotate so this core's query half sits in columns 0:HALF
            x_rot = np.concatenate([xr[b][:, HALF:], xr[b][:, :HALF]], axis=1)
        in_maps.append({
            "x_full": np.ascontiguousarray(x_rot),
            "wqkvT": wqkvT,
            "woutT": woutT,
            "bout": bout,
        })
    return in_maps


def postprocess(results):
    outs = [results[c]["out"] for c in range(NCORES)]
    full = np.stack([np.concatenate([outs[2 * b], outs[2 * b + 1]], axis=1)
                     for b in range(B)])               # [B, C, N]
    return full.reshape(B, C, 64, 64).astype(np.float32)


def kernel(x, w_qkv, w_out, b_out):
    in_maps = prepare_in_maps(x, w_qkv, w_out, b_out)
    res = run_bass_kernel_spmd(_get_nc(), in_maps, core_ids=list(range(NCORES)))
    return postprocess(res.results)


# revision 9
# speedup vs baseline: 1.0237x; 1.0237x over previous
"""Trainium2 Bass kernel for single-head self-attention over image tokens.

Reference computation (per batch element b of 4):
    xf   = x[b] viewed as [N=4096 tokens, C=256]          (x stored [C, H*W] = xf.T)
    qkv  = xf @ w_qkv.T                                   -> q, k, v each [N, 512]
    sim  = (q * 64**-0.5) @ k.T                           [N, N]
    attn = softmax(sim, axis=-1)
    out  = (attn @ v) @ w_out.T + b_out + xf              [N, C]

Sharding: 8 cores = 4 batches x 2 query-row halves (2048 rows each). Each core
computes k/v for its full batch but q/out only for its half. No collectives.
Each core's x is host-rotated so its query half is always columns 0:2048
(softmax over keys is permutation invariant, so key order doesn't matter).

Matmul operands use float32r: fp32 with the mantissa rounded to 11 bits
(round-half-even on the low 12 bits, same bit layout as fp32), which streams
1 PE column/cycle instead of 4 for plain fp32. x and the weights are
pre-rounded on the host and DMAed straight into float32r tiles; on-chip
intermediates (qT/kT/v/pT) get rounded by the PSUM->SBUF copy or activation
that produces them.

On-chip layout keeps everything in the "transposed activation" orientation so
no PE transposes are needed:
    qT [512, 2048] and kT [512, N] come straight out of the QKV projection
    (x's HBM layout [C, N] is already the rhs/lhsT the PE wants);
    v [N, 512] comes from the same projection with x slices as the stationary
    operand. simT [j, i] = kT.T @ qT, pT = exp(0.125*simT), then
    outT [d, i] += v_j.T @ pT accumulates in PSUM per 1024-column j-superblock
    and the softmax denominator l[1, i] += ones.T @ (pT pairs summed on
    GpSimd). Normalization is folded in at the end of the last superblock,
    per query slice: recip(l) via a fast Newton iteration on the DVE after a
    K=1 rank-1 broadcast matmul, multiplied into the final projection output.
"""

import numpy as np

import concourse.bacc as bacc
import concourse.tile as tile
import concourse.mybir as mybir
from concourse.bass_utils import run_bass_kernel_spmd

F32 = mybir.dt.float32
F32R = mybir.dt.float32r
Exp = mybir.ActivationFunctionType.Exp

B = 4
C = 256          # model dim (2 chunks of 128)
N = 4096         # tokens per batch (64*64)
HALF = N // 2    # query rows per core
INNER = 512      # qkv inner dim (4 chunks of 128)
SCALE = 0.125    # 64 ** -0.5

NCORES = 8
NJB = 4          # j superblocks per batch
JBW = N // NJB   # 1024 key columns per superblock
NSL = 4          # i slices per core
SW = HALF // NSL # 512 query columns per slice


def build_nc(n=N, njb=NJB, nsl=NSL):
    half = n // 2
    jbw = n // njb
    assert half % SW == 0 and jbw % SW == 0 and jbw % 256 == 0
    nc = bacc.Bacc(None)
    x_r = nc.declare_dram_parameter("x_r", [C, n], F32R, isOutput=False)
    xq_f = nc.declare_dram_parameter("xq_f", [C, half], F32, isOutput=False)
    wqkvT = nc.declare_dram_parameter("wqkvT", [C, 3 * INNER], F32R, isOutput=False)
    woutT = nc.declare_dram_parameter("woutT", [INNER, C], F32R, isOutput=False)
    bout = nc.declare_dram_parameter("bout", [2, 128, 1], F32, isOutput=False)
    out = nc.declare_dram_parameter("out", [C, half], F32, isOutput=True)

    mm = nc.tensor.matmul

    with tile.TileContext(nc) as tc:
        with tc.tile_pool(name="const", bufs=1) as const, \
             tc.tile_pool(name="stream", bufs=1) as stream, \
             tc.tile_pool(name="work", bufs=2) as work, \
             tc.tile_pool(name="pp", bufs=1, space="PSUM") as pp:

            # ---- resident weights: direct f32r DMA (host pre-rounded) ----
            wq = []
            for cc in range(2):
                t = const.tile([128, 3 * INNER], F32R, tag=f"wq{cc}", name=f"wq{cc}")
                nc.sync.dma_start(t, wqkvT[cc * 128:(cc + 1) * 128, :])
                wq.append(t)

            def xchunk(cc, col, width):
                """x chunk [128, width] in f32r, shares slots with xjb tiles."""
                t = stream.tile([128, width], F32R, tag=f"xjb{cc}", bufs=2,
                                name=f"xjb{cc}", padded_shape=[128, jbw])
                nc.sync.dma_start(t, x_r[cc * 128:(cc + 1) * 128, col:col + width])
                return t

            qT = [const.tile([128, half], F32R, tag=f"qt{d}", name=f"qt{d}")
                  for d in range(4)]
            ot = [const.tile([128, half], F32, tag=f"ot{d}", name=f"ot{d}")
                  for d in range(4)]
            l_sb = const.tile([1, half], F32, tag="l_sb", name="l_sb")

            ones_col_f = const.tile([128, 1], F32, tag="ones_col_f", name="ones_col_f")
            nc.vector.memset(ones_col_f, 1.0)
            ones_col = const.tile([128, 1], F32R, tag="ones_col", name="ones_col")
            nc.vector.tensor_copy(ones_col, ones_col_f)
            ones_row_f = const.tile([1, 128], F32, tag="ones_row_f", name="ones_row_f")
            nc.vector.memset(ones_row_f, 1.0)
            ones_row = const.tile([1, 128], F32R, tag="ones_row", name="ones_row")
            nc.vector.tensor_copy(ones_row, ones_row_f)

            # ---- qT production from x columns 0:half ----
            wo = []
            xqt = []
            bt = []
            qcw = min(1024, half)
            for qch in range(half // qcw):
                xch = [xchunk(cc, qch * qcw, qcw) for cc in range(2)]
                for d in range(4):
                    for nb in range(qcw // SW):
                        ns = qch * (qcw // SW) + nb
                        ps = pp.tile([128, SW], F32, tag="sim", bufs=2, name="ps_q")
                        for cc in range(2):
                            mm(ps, wq[cc][:, d * 128:(d + 1) * 128],
                               xch[cc][:, nb * SW:(nb + 1) * SW],
                               start=(cc == 0), stop=(cc == 1))
                        nc.scalar.copy(qT[d][:, ns * SW:(ns + 1) * SW], ps)
                if qch == 0:
                    # final-phase constants, off the startup critical path
                    for d in range(4):
                        t = const.tile([128, C], F32R, tag=f"wo{d}", name=f"wo{d}")
                        nc.gpsimd.dma_start(t, woutT[d * 128:(d + 1) * 128, :])
                        wo.append(t)
                    for cc in range(2):
                        t = const.tile([128, half], F32, tag=f"xq{cc}",
                                       name=f"xq{cc}")
                        nc.gpsimd.dma_start(
                            t, xq_f[cc * 128:(cc + 1) * 128, :])
                        xqt.append(t)
                    for cc in range(2):
                        t = const.tile([128, 1], F32, tag=f"b{cc}", name=f"b{cc}")
                        nc.gpsimd.dma_start(t, bout[cc])
                        bt.append(t)

            # residual-with-bias: xqt <- xqt + b
            for cc in range(2):
                nc.vector.tensor_scalar_add(xqt[cc], xqt[cc], bt[cc])

            # ---- attention over j superblocks ----
            for jb in range(njb):
                xjb = [xchunk(cc, jb * jbw, jbw) for cc in range(2)]
                # kT for this superblock: [512, jbw]
                kt = [stream.tile([128, jbw], F32R, tag=f"kt{d}", bufs=1,
                                  name=f"kt{d}") for d in range(4)]
                for d in range(4):
                    for nb in range(jbw // SW):
                        ps = pp.tile([128, SW], F32, tag="sim", bufs=2, name="ps_k")
                        for cc in range(2):
                            mm(ps, wq[cc][:, INNER + d * 128:INNER + (d + 1) * 128],
                               xjb[cc][:, nb * SW:(nb + 1) * SW],
                               start=(cc == 0), stop=(cc == 1))
                        nc.scalar.copy(kt[d][:, nb * SW:(nb + 1) * SW], ps)
                # v for this superblock: [jbw, 512] (token rows on partitions)
                vt = []
                for nj in range(jbw // 128):
                    t = stream.tile([128, INNER], F32R, tag=f"vt{nj}", bufs=1,
                                    name=f"vt{nj}")
                    ps = pp.tile([128, INNER], F32, tag="sim", bufs=2, name="ps_v")
                    for cc in range(2):
                        mm(ps, xjb[cc][:, nj * 128:(nj + 1) * 128],
                           wq[cc][:, 2 * INNER:3 * INNER],
                           start=(cc == 0), stop=(cc == 1))
                    nc.scalar.copy(t, ps)
                    vt.append(t)

                nj8 = jbw // 128
                for s in range(nsl):
                    sl = slice(s * SW, (s + 1) * SW)
                    po = [pp.tile([128, SW], F32, tag=f"po{d}", bufs=1,
                                  name=f"po{d}") for d in range(4)]
                    pl = pp.tile([1, SW], F32, tag="aux", bufs=2, name="pl")
                    pts = []
                    for j8 in range(nj8):
                        ps = pp.tile([128, SW], F32, tag="sim", bufs=2, name="ps_s")
                        for d in range(4):
                            mm(ps, kt[d][:, j8 * 128:(j8 + 1) * 128], qT[d][:, sl],
                               start=(d == 0), stop=(d == 3))
                        pt = work.tile([128, SW], F32R, tag="pt", bufs=4, name="pt")
                        nc.scalar.activation(pt, ps, Exp, scale=SCALE)
                        for d in range(4):
                            mm(po[d], vt[j8][:, d * 128:(d + 1) * 128], pt,
                               start=(j8 == 0), stop=(j8 == nj8 - 1))
                        pts.append(pt)
                        if j8 % 2 == 1:
                            # softmax denominator: sum pT pairs on GpSimd, then
                            # one ones-matmul per pair (halves the PE cost)
                            pt2 = work.tile([128, SW], F32R, tag="pt2", bufs=2,
                                            name="pt2")
                            nc.gpsimd.tensor_add(pt2, pts[-2], pts[-1])
                            mm(pl, ones_col, pt2,
                               start=(j8 == 1), stop=(j8 == nj8 - 1))
                    if jb == 0:
                        nc.vector.tensor_copy(l_sb[:, sl], pl)
                    else:
                        nc.vector.tensor_add(l_sb[:, sl], l_sb[:, sl], pl)
                    for d in range(4):
                        if jb == 0:
                            nc.vector.tensor_copy(ot[d][:, sl], po[d])
                        else:
                            nc.vector.tensor_add(ot[d][:, sl], ot[d][:, sl], po[d])

                    if jb == njb - 1:
                        # ---- finalize slice s: normalize + project + out ----
                        l_rs = work.tile([1, SW], F32R, tag="l_rs", bufs=2,
                                         name="l_rs")
                        nc.vector.tensor_copy(l_rs, l_sb[:, sl])
                        pb = pp.tile([128, SW], F32, tag="aux", bufs=2, name="pb")
                        mm(pb, ones_row, l_rs, start=True, stop=True)
                        bc = work.tile([128, SW], F32, tag="bc", bufs=2, name="bc")
                        rsc = work.tile([128, SW], F32, tag="rsc", bufs=2,
                                        name="rsc")
                        nc.vector.reciprocal_approx_accurate(bc, pb, rsc)
                        otr = [work.tile([128, SW], F32R, tag=f"otr{d}", bufs=1,
                                         name=f"otr{d}") for d in range(4)]
                        for d in range(4):
                            nc.scalar.copy(otr[d], ot[d][:, sl])
                        for cc in range(2):
                            pf = pp.tile([128, SW], F32, tag="aux", bufs=2,
                                         name="pf")
                            for d in range(4):
                                mm(pf, wo[d][:, cc * 128:(cc + 1) * 128], otr[d],
                                   start=(d == 0), stop=(d == 3))
                            fo = work.tile([128, SW], F32, tag="fo", bufs=2,
                                           name="fo")
                            nc.vector.tensor_mul(fo, pf, bc)
                            nc.vector.tensor_add(fo, fo, xqt[cc][:, sl])
                            nc.sync.dma_start(out[cc * 128:(cc + 1) * 128, sl], fo)

    nc.finalize()
    return nc


_NC_CACHE = None


def _get_nc():
    global _NC_CACHE
    if _NC_CACHE is None:
        _NC_CACHE = build_nc()
    return _NC_CACHE


def _round_f32r(a):
    """fp32 -> float32r rounding (round-half-even on the low 12 mantissa
    bits), matching the hardware's fp32_to_fp32r conversion."""
    bits = np.ascontiguousarray(a, dtype=np.float32).view(np.uint32)
    rem = bits & np.uint32(0xFFF)
    base = bits & np.uint32(0xFFFFF000)
    up = (rem > 0x800) | ((rem == 0x800) & (((bits >> np.uint32(12)) & np.uint32(1)) == 1))
    return (base + np.where(up, np.uint32(0x1000), np.uint32(0))).view(np.float32)


def prepare_in_maps(x, w_qkv, w_out, b_out):
    x = np.asarray(x, dtype=np.float32)
    w_qkv = np.asarray(w_qkv, dtype=np.float32)
    w_out = np.asarray(w_out, dtype=np.float32)
    b_out = np.asarray(b_out, dtype=np.float32)

    xr = x.reshape(B, C, N)
    wqkvT = _round_f32r(np.ascontiguousarray(w_qkv.T))   # [C, 1536]
    woutT = _round_f32r(np.ascontiguousarray(w_out.T))   # [512, C]
    bout = np.ascontiguousarray(b_out.reshape(2, 128, 1))

    in_maps = []
    for c in range(NCORES):
        b, h = divmod(c, 2)
        if h == 0:
            x_rot = xr[b]
        else:  # rotate so this core's query half sits in columns 0:HALF
            x_rot = np.concatenate([xr[b][:, HALF:], xr[b][:, :HALF]], axis=1)
        in_maps.append({
            "x_r": _round_f32r(x_rot),
            "xq_f": np.ascontiguousarray(x_rot[:, :HALF]),
            "wqkvT": wqkvT,
            "woutT": woutT,
            "bout": bout,
        })
    return in_maps


def postprocess(results):
    outs = [results[c]["out"] for c in range(NCORES)]
    full = np.stack([np.concatenate([outs[2 * b], outs[2 * b + 1]], axis=1)
                     for b in range(B)])               # [B, C, N]
    return full.reshape(B, C, 64, 64).astype(np.float32)


def kernel(x, w_qkv, w_out, b_out):
    in_maps = prepare_in_maps(x, w_qkv, w_out, b_out)
    res = run_bass_kernel_spmd(_get_nc(), in_maps, core_ids=list(range(NCORES)))
    return postprocess(res.results)


# revision 11
# speedup vs baseline: 1.1122x; 1.0865x over previous
"""Trainium2 Bass kernel for single-head self-attention over image tokens.

Reference computation (per batch element b of 4):
    xf   = x[b] viewed as [N=4096 tokens, C=256]          (x stored [C, H*W] = xf.T)
    qkv  = xf @ w_qkv.T                                   -> q, k, v each [N, 512]
    sim  = (q * 64**-0.5) @ k.T                           [N, N]
    attn = softmax(sim, axis=-1)
    out  = (attn @ v) @ w_out.T + b_out + xf              [N, C]

Sharding: 8 cores = 4 batches x 2 query-row halves (2048 rows each). Each core
computes k/v for its full batch but q/out only for its half. No collectives.
Each core's x is host-rotated so its query half is always columns 0:2048
(softmax over keys is permutation invariant, so key order doesn't matter).

Matmul operands use float32r: fp32 with the mantissa rounded to 11 bits
(round-half-even on the low 12 bits, same bit layout as fp32), which streams
1 PE column/cycle instead of 4 for plain fp32. x and the weights are
pre-rounded on the host and DMAed straight into float32r tiles; on-chip
intermediates (qT/kT/v/pT) get rounded by the PSUM->SBUF copy or activation
that produces them.

On-chip layout keeps everything in the "transposed activation" orientation so
no PE transposes are needed:
    qT [512, 2048] and kT [512, N] come straight out of the QKV projection
    (x's HBM layout [C, N] is already the rhs/lhsT the PE wants);
    v [N, 512] comes from the same projection with x slices as the stationary
    operand. simT [j, i] = kT.T @ qT, pT = exp(0.125*simT), then
    outT [d, i] += v_j.T @ pT accumulates in PSUM per 1024-column j-superblock
    and the softmax denominator l[1, i] += ones.T @ (pT pairs summed on
    GpSimd). Normalization is folded in at the end of the last superblock,
    per query slice: recip(l) via a fast Newton iteration on the DVE after a
    K=1 rank-1 broadcast matmul, multiplied into the final projection output.
"""

import numpy as np

import concourse.bacc as bacc
import concourse.tile as tile
import concourse.mybir as mybir
from concourse.bass_utils import run_bass_kernel_spmd

F32 = mybir.dt.float32
F32R = mybir.dt.float32r
Exp = mybir.ActivationFunctionType.Exp

B = 4
C = 256          # model dim (2 chunks of 128)
N = 4096         # tokens per batch (64*64)
HALF = N // 2    # query rows per core
INNER = 512      # qkv inner dim (4 chunks of 128)
SCALE = 0.125    # 64 ** -0.5

NCORES = 8
NJB = 4          # j superblocks per batch
JBW = N // NJB   # 1024 key columns per superblock
NSL = 4          # i slices per core
SW = HALF // NSL # 512 query columns per slice


def build_nc(n=N, njb=NJB, nsl=NSL):
    half = n // 2
    jbw = n // njb
    assert half % SW == 0 and jbw % SW == 0 and jbw % 256 == 0
    nc = bacc.Bacc(None)
    x_r = nc.declare_dram_parameter("x_r", [C, n], F32R, isOutput=False)
    xq_f = nc.declare_dram_parameter("xq_f", [C, half], F32, isOutput=False)
    wqkvT = nc.declare_dram_parameter("wqkvT", [C, 3 * INNER], F32R, isOutput=False)
    woutT = nc.declare_dram_parameter("woutT", [INNER, C], F32R, isOutput=False)
    bout = nc.declare_dram_parameter("bout", [2, 128, 1], F32, isOutput=False)
    out = nc.declare_dram_parameter("out", [C, half], F32, isOutput=True)

    mm = nc.tensor.matmul

    with tile.TileContext(nc) as tc:
        with tc.tile_pool(name="const", bufs=1) as const, \
             tc.tile_pool(name="stream", bufs=1) as stream, \
             tc.tile_pool(name="work", bufs=2) as work, \
             tc.tile_pool(name="pp", bufs=1, space="PSUM") as pp:

            # ---- resident weights: direct f32r DMA (host pre-rounded) ----
            wq = []
            for cc in range(2):
                t = const.tile([128, 3 * INNER], F32R, tag=f"wq{cc}", name=f"wq{cc}")
                nc.sync.dma_start(t, wqkvT[cc * 128:(cc + 1) * 128, :])
                wq.append(t)

            def xchunk(cc, col, width):
                """x chunk [128, width] in f32r, shares slots with xjb tiles."""
                t = stream.tile([128, width], F32R, tag=f"xjb{cc}", bufs=2,
                                name=f"xjb{cc}", padded_shape=[128, jbw])
                nc.sync.dma_start(t, x_r[cc * 128:(cc + 1) * 128, col:col + width])
                return t

            qT = [const.tile([128, half], F32R, tag=f"qt{d}", name=f"qt{d}")
                  for d in range(4)]
            ot = [const.tile([128, half], F32, tag=f"ot{d}", name=f"ot{d}")
                  for d in range(4)]
            l_sb = const.tile([1, half], F32, tag="l_sb", name="l_sb")

            ones_col_f = const.tile([128, 1], F32, tag="ones_col_f", name="ones_col_f")
            nc.vector.memset(ones_col_f, 1.0)
            ones_col = const.tile([128, 1], F32R, tag="ones_col", name="ones_col")
            nc.vector.tensor_copy(ones_col, ones_col_f)
            ones_row_f = const.tile([1, 128], F32, tag="ones_row_f", name="ones_row_f")
            nc.vector.memset(ones_row_f, 1.0)
            ones_row = const.tile([1, 128], F32R, tag="ones_row", name="ones_row")
            nc.vector.tensor_copy(ones_row, ones_row_f)

            # ---- qT production from x columns 0:half ----
            wo = []
            xqt = []
            bt = []
            qcw = min(1024, half)
            for qch in range(half // qcw):
                xch = [xchunk(cc, qch * qcw, qcw) for cc in range(2)]
                for d in range(4):
                    for nb in range(qcw // SW):
                        ns = qch * (qcw // SW) + nb
                        ps = pp.tile([128, SW], F32, tag="sim", bufs=2, name="ps_q")
                        for cc in range(2):
                            mm(ps, wq[cc][:, d * 128:(d + 1) * 128],
                               xch[cc][:, nb * SW:(nb + 1) * SW],
                               start=(cc == 0), stop=(cc == 1))
                        nc.scalar.copy(qT[d][:, ns * SW:(ns + 1) * SW], ps)
            # final-phase constants, off the startup critical path
            # (vector-queue DMAs so the sync queue stays free for x chunks)
            for d in range(4):
                t = const.tile([128, C], F32R, tag=f"wo{d}", name=f"wo{d}")
                nc.scalar.dma_start(t, woutT[d * 128:(d + 1) * 128, :])
                wo.append(t)
            for cc in range(2):
                t = const.tile([128, half], F32, tag=f"xq{cc}", name=f"xq{cc}")
                nc.scalar.dma_start(t, xq_f[cc * 128:(cc + 1) * 128, :])
                xqt.append(t)
            for cc in range(2):
                t = const.tile([128, 1], F32, tag=f"b{cc}", name=f"b{cc}")
                nc.scalar.dma_start(t, bout[cc])
                bt.append(t)

            # residual-with-bias: xqt <- xqt + b
            for cc in range(2):
                nc.vector.tensor_scalar_add(xqt[cc], xqt[cc], bt[cc])

            # ---- attention over j superblocks ----
            for jb in range(njb):
                xjb = [xchunk(cc, jb * jbw, jbw) for cc in range(2)]
                # kT for this superblock: [512, jbw]
                kt = [stream.tile([128, jbw], F32R, tag=f"kt{d}", bufs=1,
                                  name=f"kt{d}") for d in range(4)]
                for d in range(4):
                    for nb in range(jbw // SW):
                        ps = pp.tile([128, SW], F32, tag="sim", bufs=2, name="ps_k")
                        for cc in range(2):
                            mm(ps, wq[cc][:, INNER + d * 128:INNER + (d + 1) * 128],
                               xjb[cc][:, nb * SW:(nb + 1) * SW],
                               start=(cc == 0), stop=(cc == 1))
                        nc.scalar.copy(kt[d][:, nb * SW:(nb + 1) * SW], ps)
                # v for this superblock: [jbw, 512] (token rows on partitions)
                vt = []
                for nj in range(jbw // 128):
                    t = stream.tile([128, INNER], F32R, tag=f"vt{nj}", bufs=1,
                                    name=f"vt{nj}")
                    ps = pp.tile([128, INNER], F32, tag="sim", bufs=2, name="ps_v")
                    for cc in range(2):
                        mm(ps, xjb[cc][:, nj * 128:(nj + 1) * 128],
                           wq[cc][:, 2 * INNER:3 * INNER],
                           start=(cc == 0), stop=(cc == 1))
                    nc.scalar.copy(t, ps)
                    vt.append(t)

                nj8 = jbw // 128
                for s in range(nsl):
                    sl = slice(s * SW, (s + 1) * SW)
                    po = [pp.tile([128, SW], F32, tag=f"po{d}", bufs=1,
                                  name=f"po{d}") for d in range(4)]
                    pl = pp.tile([1, SW], F32, tag="aux", bufs=2, name="pl")
                    pts = []

                    def drain_j8(j8):
                        # outT + denominator work for chunk j8 (emitted one
                        # chunk late so the PE never waits on the exp)
                        pt = pts[j8]
                        for d in range(4):
                            mm(po[d], vt[j8][:, d * 128:(d + 1) * 128], pt,
                               start=(j8 == 0), stop=(j8 == nj8 - 1))
                        if j8 % 2 == 1:
                            # denominator: sum pT pairs on DVE, then one
                            # ones-matmul per pair (halves the PE cost)
                            pt2 = work.tile([128, SW], F32R, tag="pt2", bufs=2,
                                            name="pt2")
                            nc.vector.tensor_add(pt2, pts[j8 - 1], pt)
                            mm(pl, ones_col, pt2,
                               start=(j8 == 1), stop=(j8 == nj8 - 1))

                    for j8 in range(nj8):
                        ps = pp.tile([128, SW], F32, tag="sim", bufs=2, name="ps_s")
                        for d in range(4):
                            mm(ps, kt[d][:, j8 * 128:(j8 + 1) * 128], qT[d][:, sl],
                               start=(d == 0), stop=(d == 3))
                        pt = work.tile([128, SW], F32R, tag="pt", bufs=4, name="pt")
                        nc.scalar.activation(pt, ps, Exp, scale=SCALE)
                        pts.append(pt)
                        if j8 > 0:
                            drain_j8(j8 - 1)
                    drain_j8(nj8 - 1)
                    if jb == 0:
                        nc.vector.tensor_copy(l_sb[:, sl], pl)
                    else:
                        nc.vector.tensor_add(l_sb[:, sl], l_sb[:, sl], pl)
                    for d in range(4):
                        if jb == 0:
                            nc.vector.tensor_copy(ot[d][:, sl], po[d])
                        else:
                            nc.vector.tensor_add(ot[d][:, sl], ot[d][:, sl], po[d])

                    if jb == njb - 1:
                        # ---- finalize slice s: normalize + project + out ----
                        l_rs = work.tile([1, SW], F32R, tag="l_rs", bufs=2,
                                         name="l_rs")
                        nc.scalar.copy(l_rs, l_sb[:, sl])
                        pb = pp.tile([128, SW], F32, tag="aux", bufs=2, name="pb")
                        mm(pb, ones_row, l_rs, start=True, stop=True)
                        bc = work.tile([128, SW], F32, tag="bc", bufs=2, name="bc")
                        rsc = work.tile([128, SW], F32, tag="rsc", bufs=2,
                                        name="rsc")
                        nc.vector.reciprocal_approx_accurate(bc, pb, rsc)
                        otr = [work.tile([128, SW], F32R, tag=f"otr{d}", bufs=1,
                                         name=f"otr{d}") for d in range(4)]
                        for d in range(4):
                            nc.scalar.copy(otr[d], ot[d][:, sl])
                        for cc in range(2):
                            pf = pp.tile([128, SW], F32, tag="aux", bufs=2,
                                         name="pf")
                            for d in range(4):
                                mm(pf, wo[d][:, cc * 128:(cc + 1) * 128], otr[d],
                                   start=(d == 0), stop=(d == 3))
                            fo = work.tile([128, SW], F32, tag="fo", bufs=2,
                                           name="fo")
                            nc.vector.tensor_mul(fo, pf, bc)
                            nc.vector.tensor_add(fo, fo, xqt[cc][:, sl])
                            nc.sync.dma_start(out[cc * 128:(cc + 1) * 128, sl], fo)

    nc.finalize()
    return nc


_NC_CACHE = None


def _get_nc():
    global _NC_CACHE
    if _NC_CACHE is None:
        _NC_CACHE = build_nc()
    return _NC_CACHE


def _round_f32r(a):
    """fp32 -> float32r rounding (round-half-even on the low 12 mantissa
    bits), matching the hardware's fp32_to_fp32r conversion."""
    bits = np.ascontiguousarray(a, dtype=np.float32).view(np.uint32)
    rem = bits & np.uint32(0xFFF)
    base = bits & np.uint32(0xFFFFF000)
    up = (rem > 0x800) | ((rem == 0x800) & (((bits >> np.uint32(12)) & np.uint32(1)) == 1))
    return (base + np.where(up, np.uint32(0x1000), np.uint32(0))).view(np.float32)


def prepare_in_maps(x, w_qkv, w_out, b_out):
    x = np.asarray(x, dtype=np.float32)
    w_qkv = np.asarray(w_qkv, dtype=np.float32)
    w_out = np.asarray(w_out, dtype=np.float32)
    b_out = np.asarray(b_out, dtype=np.float32)

    xr = x.reshape(B, C, N)
    wqkvT = _round_f32r(np.ascontiguousarray(w_qkv.T))   # [C, 1536]
    woutT = _round_f32r(np.ascontiguousarray(w_out.T))   # [512, C]
    bout = np.ascontiguousarray(b_out.reshape(2, 128, 1))

    in_maps = []
    for c in range(NCORES):
        b, h = divmod(c, 2)
        if h == 0:
            x_rot = xr[b]
        else:  # rotate so this core's query half sits in columns 0:HALF
            x_rot = np.concatenate([xr[b][:, HALF:], xr[b][:, :HALF]], axis=1)
        in_maps.append({
            "x_r": _round_f32r(x_rot),
            "xq_f": np.ascontiguousarray(x_rot[:, :HALF]),
            "wqkvT": wqkvT,
            "woutT": woutT,
            "bout": bout,
        })
    return in_maps


def postprocess(results):
    outs = [results[c]["out"] for c in range(NCORES)]
    full = np.stack([np.concatenate([outs[2 * b], outs[2 * b + 1]], axis=1)
                     for b in range(B)])               # [B, C, N]
    return full.reshape(B, C, 64, 64).astype(np.float32)


def kernel(x, w_qkv, w_out, b_out):
    in_maps = prepare_in_maps(x, w_qkv, w_out, b_out)
    res = run_bass_kernel_spmd(_get_nc(), in_maps, core_ids=list(range(NCORES)))
    return postprocess(res.results)
